# revision 14
# baseline (speedup 1.0000x reference)
"""Trainium2 Bass kernel for the CARU decoder (nn_Decoder_22737556865467).

Math (per step t, teacher forcing):
    w      = emb[words[t]]                  # [B, EMB]
    feat   = w @ W_lin + b_lin              # [B, HID]
    n      = tanh(h @ W_w + b_w + feat)
    l      = sigmoid(feat) * sigmoid(h @ W_lw + b_lw + w @ W_ll + b_ll)
    h      = h + l * (n - h)
    out[t] = h @ W_out + b_out              # [B, VOCAB]

Distribution: the recurrence (small, strictly sequential) is replicated on all
8 cores; the output projection and W_out are sharded over the vocab dimension
(4000 columns per core, padded to 4096).

Device program (SPMD, identical on every core except the W_out shard):
  phase 0: F = x@W_lin + b_lin (+b_w variant), S = sigmoid(F), G = x@W_ll +
           b_ll + b_lw for all T*B rows -> DRAM scratch (bf16 matmuls).
  phase A: 80 sequential CARU steps.  h lives in a packed [128, 512] layout
           (partition = 4 hid-chunks x 32 batch, free = 512 hid).  The two
           [32,2048]x[2048,2048] matmuls per step use h^T tiles [128, 32] as
           the PE stationary operand, 4-way column-packed over the PE array
           (tile_position), with bf16 [W_w | W_lw] as the moving operand.
           TensorE transposes rebuild h^T after each update; h^T (bf16) is
           streamed to DRAM for phase B.
  phase B: projection H[2560,2048] @ W_out[2048, 4096-shard].  Stationary =
           h^T chunk tiles [128, 128], moving = resident bf16 W_out, 8 PSUM
           banks accumulating over the 16 K-tiles.
"""

import os
import sys

import numpy as np

VOCAB, EMB, HID, T, B = 32000, 256, 2048, 80, 32
NCORES = 8
VSH = VOCAB // NCORES  # 4000
VPAD = 4096
UNK = 0


def _ensure_concourse():
    try:
        import concourse.bass  # noqa: F401
        return
    except ImportError:
        pass
    for p in ("/opt/trn_rl_repo", "/root/.axon_site/_ro/trn_rl_repo"):
        if os.path.isdir(p) and p not in sys.path:
            sys.path.insert(0, p)
    import concourse.bass  # noqa: F401


def build_decoder(nc, tc, Tn, vpad, has_bias, has_bw):
    """Emit the decoder program into TileContext tc. Returns nothing; tensors
    are declared by name (see in-map construction below)."""
    from contextlib import ExitStack

    import concourse.mybir as mybir

    f32 = mybir.dt.float32
    bf16 = mybir.dt.bfloat16
    AF = mybir.ActivationFunctionType

    rows = Tn * 32
    assert rows % 128 == 0
    n_rt = rows // 128
    KE = EMB // 128  # k-tiles over EMB (2)
    NCH = HID // 512  # 512-chunks over HID (4)
    KT = HID // 128  # k-tiles over HID (16)
    NB = vpad // 512  # psum banks in phase B
    assert NB <= 8
    cs = 16 if Tn % 16 == 0 else Tn  # chunk steps for phase B
    assert Tn % cs == 0 and cs % 4 == 0
    n_ck = Tn // cs
    n_mt = cs // 4  # 128-row tiles per chunk

    xT = nc.dram_tensor("x_T", [EMB, rows], bf16, kind="ExternalInput").ap()
    wlin = nc.dram_tensor("W_lin", [EMB, HID], bf16, kind="ExternalInput").ap()
    wll = nc.dram_tensor("W_ll", [EMB, HID], bf16, kind="ExternalInput").ap()
    hpk = nc.dram_tensor("h_pack", [128, 512], f32, kind="ExternalInput").ap()
    ht0 = nc.dram_tensor("hT0", [128, 512], bf16, kind="ExternalInput").ap()
    wc = nc.dram_tensor("Wc", [HID, 2 * HID], bf16, kind="ExternalInput").ap()
    wout = nc.dram_tensor("Wout", [HID, vpad], bf16, kind="ExternalInput").ap()
    idin = nc.dram_tensor("ident", [128, 128], f32, kind="ExternalInput").ap()
    bia = nc.dram_tensor("biasv", [3, HID], bf16, kind="ExternalInput").ap()
    out = nc.dram_tensor("out", [rows, vpad], f32, kind="ExternalOutput").ap()

    with ExitStack() as ctx:
        constp = ctx.enter_context(tc.tile_pool(name="const", bufs=1))
        dramp = ctx.enter_context(tc.tile_pool(name="drsc", bufs=1, space="DRAM"))

        ident = constp.tile([128, 128], f32)
        nc.sync.dma_start(ident[:], idin)
        zeros_bf = constp.tile([128, 128], bf16)
        nc.vector.memset(zeros_bf[:], 0.0)
        ones_sb = constp.tile([1, 128], bf16)
        bias_sb = constp.tile([3, HID], bf16)
        if has_bias:
            nc.vector.memset(ones_sb[:], 1.0)
            nc.sync.dma_start(bias_sb[:], bia)

        F2d = dramp.tile([rows, HID], f32)
        Sd = dramp.tile([rows, HID], f32)
        Gd = dramp.tile([rows, HID], f32)
        HTd = dramp.tile([Tn, 128, 512], bf16)

        # ---------------- phase 0: F2/S/G precompute ----------------
        with tc.tile_pool(name="p0w", bufs=1) as p0w, \
                tc.tile_pool(name="p0io", bufs=4) as p0io, \
                tc.tile_pool(name="p0ps", bufs=4, space="PSUM") as p0ps:
            xTs = p0w.tile([128, KE * rows], bf16)
            for k in range(KE):
                nc.sync.dma_start(
                    xTs[:, k * rows:(k + 1) * rows], xT[k * 128:(k + 1) * 128, :])
            wls = p0w.tile([128, KE * HID], bf16)
            wlls = p0w.tile([128, KE * HID], bf16)
            for k in range(KE):
                nc.sync.dma_start(
                    wls[:, k * HID:(k + 1) * HID], wlin[k * 128:(k + 1) * 128, :])
                nc.sync.dma_start(
                    wlls[:, k * HID:(k + 1) * HID], wll[k * 128:(k + 1) * 128, :])

            for rt in range(n_rt):
                for ncx in range(NCH):
                    sl = slice(ncx * 512, ncx * 512 + 512)
                    psF = p0ps.tile([128, 512], f32, tag="psF")
                    psG = p0ps.tile([128, 512], f32, tag="psG")
                    for k in range(KE):
                        lh = xTs[:, k * rows + rt * 128:
                                 k * rows + rt * 128 + 128]
                        nc.tensor.matmul(
                            psF[:], lh,
                            wls[:, k * HID + ncx * 512:
                                k * HID + ncx * 512 + 512],
                            start=(k == 0), stop=(k == KE - 1 and not has_bias))
                        nc.tensor.matmul(
                            psG[:], lh,
                            wlls[:, k * HID + ncx * 512:
                                 k * HID + ncx * 512 + 512],
                            start=(k == 0), stop=(k == KE - 1 and not has_bias))
                    if has_bias:
                        nc.tensor.matmul(
                            psF[:], ones_sb[0:1, :],
                            bias_sb[0:1, sl],
                            start=False, stop=True)
                        nc.tensor.matmul(
                            psG[:], ones_sb[0:1, :],
                            bias_sb[2:3, sl],
                            start=False, stop=True)
                    S_t = p0io.tile([128, 512], f32, tag="S")
                    nc.scalar.activation(S_t[:], psF[:], AF.Sigmoid)
                    if has_bw:
                        # F2 = F + b_w (tanh input); must come after S reads psF
                        nc.tensor.matmul(
                            psF[:], ones_sb[0:1, :],
                            bias_sb[1:2, sl],
                            start=False, stop=True, skip_group_check=True)
                    F2_t = p0io.tile([128, 512], f32, tag="F2")
                    nc.vector.tensor_copy(F2_t[:], psF[:])
                    G_t = p0io.tile([128, 512], f32, tag="G")
                    nc.vector.tensor_copy(G_t[:], psG[:])
                    rsl = slice(rt * 128, rt * 128 + 128)
                    nc.sync.dma_start(Sd[rsl, sl], S_t[:])
                    nc.sync.dma_start(F2d[rsl, sl], F2_t[:])
                    nc.sync.dma_start(Gd[rsl, sl], G_t[:])

        # ---------------- phase A: recurrence ----------------
        with tc.tile_pool(name="wcp", bufs=1) as wcp, \
                tc.tile_pool(name="hp", bufs=2) as hp, \
                tc.tile_pool(name="htp", bufs=2) as htp, \
                tc.tile_pool(name="stp", bufs=3) as stp, \
                tc.tile_pool(name="ewp", bufs=2) as ewp, \
                tc.tile_pool(name="apsp", bufs=2, space="PSUM") as apsp, \
                tc.tile_pool(name="trpp", bufs=2, space="PSUM") as trpp:
            wcs = [wcp.tile([128, 8 * 4096], bf16, tag=f"wc{i}", name=f"wc{i}")
                   for i in range(2)]
            for kt in range(KT):
                nc.sync.dma_start(
                    wcs[kt // 8][:, (kt % 8) * 4096:(kt % 8 + 1) * 4096],
                    wc[kt * 128:(kt + 1) * 128, :])

            h_cur = hp.tile([128, 512], f32, tag="h")
            nc.sync.dma_start(h_cur[:], hpk)
            hT_cur = htp.tile([128, 512], bf16, tag="hT")
            nc.sync.dma_start(hT_cur[:], ht0)

            for t in range(Tn):
                F2_t = stp.tile([128, 512], f32, tag="f2")
                S_t = stp.tile([128, 512], f32, tag="s")
                G_t = stp.tile([128, 512], f32, tag="g")
                src = slice(t * 32, (t + 1) * 32)
                nc.sync.dma_start(
                    F2_t[:], F2d[src, :].rearrange("b (c f) -> c b f", c=4))
                nc.sync.dma_start(
                    S_t[:], Sd[src, :].rearrange("b (c f) -> c b f", c=4))
                nc.sync.dma_start(
                    G_t[:], Gd[src, :].rearrange("b (c f) -> c b f", c=4))

                ps0 = apsp.tile([128, 512], f32, tag="ps0")
                ps1 = apsp.tile([128, 512], f32, tag="ps1")
                # Initialize both banks with a full-partition zero matmul so the
                # partition-sliced column-group matmuls can all accumulate
                # (start=False) — safe under both whole-bank and per-partition
                # has_written-clear semantics.
                nc.tensor.matmul(ps1[:], zeros_bf[:], wcs[0][:, 0:512],
                                 start=True, stop=False)
                nc.tensor.matmul(ps0[:], zeros_bf[:], wcs[0][:, 0:512],
                                 start=True, stop=False)
                for kt in range(KT):
                    off = 128 * (kt % 4) + 32 * (kt // 4)
                    lh = hT_cur[:, off:off + 32]
                    wt = wcs[kt // 8]
                    base = (kt % 8) * 4096
                    for g in range(4):
                        gs = slice(32 * g, 32 * g + 32)
                        nc.tensor.matmul(
                            ps1[gs, :], lh,
                            wt[:, base + 2048 + g * 512:base + 2048 + g * 512 + 512],
                            start=False, stop=False,
                            tile_position=(0, 32 * g))
                        nc.tensor.matmul(
                            ps0[gs, :], lh,
                            wt[:, base + g * 512:base + g * 512 + 512],
                            start=False, stop=False,
                            tile_position=(0, 32 * g))
                # Close both accumulation groups with full-partition zero
                # matmuls (region-global group bookkeeping; adds 0 to the data).
                nc.tensor.matmul(ps1[:], zeros_bf[:], wcs[0][:, 0:512],
                                 start=False, stop=True)
                nc.tensor.matmul(ps0[:], zeros_bf[:], wcs[0][:, 0:512],
                                 start=False, stop=True)

                tmp1 = ewp.tile([128, 512], f32, tag="t1")
                nc.vector.tensor_add(tmp1[:], ps1[:], G_t[:])
                s1 = ewp.tile([128, 512], f32, tag="s1")
                nc.scalar.activation(s1[:], tmp1[:], AF.Sigmoid)
                l_ = ewp.tile([128, 512], f32, tag="l")
                nc.vector.tensor_mul(l_[:], S_t[:], s1[:])
                lh_ = ewp.tile([128, 512], f32, tag="lh")
                nc.vector.tensor_mul(lh_[:], l_[:], h_cur[:])
                q_ = ewp.tile([128, 512], f32, tag="q")
                nc.vector.tensor_sub(q_[:], h_cur[:], lh_[:])
                tmp0 = ewp.tile([128, 512], f32, tag="t0")
                nc.vector.tensor_add(tmp0[:], ps0[:], F2_t[:])
                n_ = ewp.tile([128, 512], f32, tag="n")
                nc.scalar.activation(n_[:], tmp0[:], AF.Tanh)
                p_ = ewp.tile([128, 512], f32, tag="p")
                nc.vector.tensor_mul(p_[:], l_[:], n_[:])
                h_new = hp.tile([128, 512], f32, tag="h")
                nc.vector.tensor_add(h_new[:], q_[:], p_[:])

                ptr = trpp.tile([128, 512], f32, tag="tr")
                for q4 in range(4):
                    nc.tensor.transpose(
                        ptr[:, 128 * q4:128 * q4 + 128],
                        h_new[:, 128 * q4:128 * q4 + 128], ident[:])
                hT_new = htp.tile([128, 512], bf16, tag="hT")
                nc.vector.tensor_copy(hT_new[:], ptr[:])
                nc.sync.dma_start(HTd[t], hT_new[:])

                h_cur, hT_cur = h_new, hT_new

        # ---------------- phase B: output projection ----------------
        with tc.tile_pool(name="wop", bufs=1) as wop, \
                tc.tile_pool(name="htcp", bufs=2) as htcp, \
                tc.tile_pool(name="obp", bufs=12) as obp, \
                tc.tile_pool(name="bpsp", bufs=NB, space="PSUM") as bpsp:
            wos = [wop.tile([128, 8 * vpad], bf16, tag=f"wo{i}", name=f"wo{i}")
                   for i in range(2)]
            for kt in range(KT):
                nc.sync.dma_start(
                    wos[kt // 8][:, (kt % 8) * vpad:(kt % 8 + 1) * vpad],
                    wout[kt * 128:(kt + 1) * 128, :])
            for ck in range(n_ck):
                # Reorder h^T into K-tile-major contiguous layout: column
                # kt*cs*32 + s*32 + b  <-  HTd[ck*cs+s, p, 128*(kt%4)+32*(kt//4)+b]
                htc = htcp.tile([128, KT * cs * 32], bf16, tag="htc")
                for kt in range(KT):
                    off = 128 * (kt % 4) + 32 * (kt // 4)
                    nc.sync.dma_start(
                        htc[:, kt * cs * 32:(kt + 1) * cs * 32],
                        HTd[ck * cs:(ck + 1) * cs, :, off:off + 32]
                        .rearrange("s p b -> p s b"))
                for mt in range(n_mt):
                    pns = [bpsp.tile([128, 512], f32, tag="pn", name=f"pn{n}")
                           for n in range(NB)]
                    for kt in range(KT):
                        lh = htc[:, kt * cs * 32 + mt * 128:
                                 kt * cs * 32 + mt * 128 + 128]
                        wtile = wos[kt // 8]
                        base = (kt % 8) * vpad
                        for n in range(NB):
                            nc.tensor.matmul(
                                pns[n][:], lh,
                                wtile[:, base + n * 512:base + (n + 1) * 512],
                                start=(kt == 0), stop=(kt == KT - 1))
                    row0 = ck * cs * 32 + mt * 128
                    for n in range(NB):
                        ob = obp.tile([128, 512], f32, tag="ob")
                        nc.vector.tensor_copy(ob[:], pns[n][:])
                        nc.sync.dma_start(
                            out[row0:row0 + 128, n * 512:(n + 1) * 512], ob[:])


def host_prepare(hidden, trg, emb, W_lin, b_lin, W_w, b_w, W_lw, b_lw, W_ll,
                 b_ll, W_out, b_out, Tn=T, vpad=VPAD, ncores=NCORES):
    """Build the per-core input maps (numpy only: gather, transpose, pack)."""
    import ml_dtypes

    bf16 = ml_dtypes.bfloat16
    trg = np.asarray(trg)
    words = np.concatenate(
        [np.full((1, trg.shape[1]), UNK, dtype=trg.dtype), trg[:-1]], axis=0)
    x = np.asarray(emb, np.float32)[words]  # [Tn, B, EMB]
    rows = Tn * B
    xT = np.ascontiguousarray(x.reshape(rows, EMB).T)

    h = np.asarray(hidden, np.float32)
    h_pack = np.ascontiguousarray(
        h.reshape(B, 4, 512).transpose(1, 0, 2).reshape(128, 512))
    hT0 = np.ascontiguousarray(
        h.reshape(B, 4, 4, 128).transpose(3, 2, 1, 0).reshape(128, 512)
    ).astype(bf16)

    Wc = np.concatenate(
        [np.asarray(W_w, np.float32), np.asarray(W_lw, np.float32)],
        axis=1).astype(bf16)
    biasv = np.stack([
        np.asarray(b_lin, np.float32),
        np.asarray(b_w, np.float32),
        np.asarray(b_lw, np.float32) + np.asarray(b_ll, np.float32)])
    ident = np.eye(128, dtype=np.float32)

    common = {
        "x_T": xT.astype(bf16),
        "W_lin": np.ascontiguousarray(np.asarray(W_lin, np.float32)).astype(bf16),
        "W_ll": np.ascontiguousarray(np.asarray(W_ll, np.float32)).astype(bf16),
        "h_pack": h_pack,
        "hT0": hT0,
        "Wc": Wc,
        "ident": ident,
        "biasv": biasv.astype(bf16),
    }
    Wo = np.asarray(W_out, np.float32)
    vsh = Wo.shape[1] // ncores
    in_maps = []
    for j in range(ncores):
        wo_j = np.zeros((HID, vpad), dtype=bf16)
        wo_j[:, :vsh] = Wo[:, j * vsh:(j + 1) * vsh].astype(bf16)
        m = dict(common)
        m["Wout"] = wo_j
        in_maps.append(m)
    has_bias = bool(np.any(biasv))
    has_bw = bool(np.any(np.asarray(b_w)))
    return in_maps, has_bias, has_bw, vsh


_CACHE = {}


def compile_decoder(Tn, vpad, has_bias, has_bw, ncores):
    key = (Tn, vpad, has_bias, has_bw, ncores)
    if key in _CACHE:
        return _CACHE[key]
    _ensure_concourse()
    import concourse.bacc as bacc
    import concourse.tile as tile

    nc = bacc.Bacc("TRN2", target_bir_lowering=False, debug=False,
                   num_devices=ncores)
    with tile.TileContext(nc) as tc:
        build_decoder(nc, tc, Tn, vpad, has_bias, has_bw)
    nc.compile()
    _CACHE[key] = nc
    return nc


def kernel(hidden, trg, emb, W_lin, b_lin, W_w, b_w, W_lw, b_lw, W_ll, b_ll,
           W_out, b_out, _trace=False):
    _ensure_concourse()
    from concourse.bass_utils import run_bass_kernel_spmd

    in_maps, has_bias, has_bw, vsh = host_prepare(
        hidden, trg, emb, W_lin, b_lin, W_w, b_w, W_lw, b_lw, W_ll, b_ll,
        W_out, b_out)
    nc = compile_decoder(T, VPAD, has_bias, has_bw, NCORES)
    res = run_bass_kernel_spmd(
        nc, in_maps, core_ids=list(range(NCORES)), trace=_trace)
    parts = [res.results[j]["out"][:, :vsh] for j in range(NCORES)]
    full = np.concatenate(parts, axis=1).reshape(T, B, VOCAB)
    b_out = np.asarray(b_out, np.float32)
    if np.any(b_out):
        full = full + b_out
    if _trace:
        kernel._last_results = res
    return np.ascontiguousarray(full.astype(np.float32))


# revision 15
# speedup vs baseline: 1.0400x; 1.0400x over previous
"""Trainium2 Bass kernel for the CARU decoder (nn_Decoder_22737556865467).

Math (per step t, teacher forcing):
    w      = emb[words[t]]                  # [B, EMB]
    feat   = w @ W_lin + b_lin              # [B, HID]
    n      = tanh(h @ W_w + b_w + feat)
    l      = sigmoid(feat) * sigmoid(h @ W_lw + b_lw + w @ W_ll + b_ll)
    h      = h + l * (n - h)
    out[t] = h @ W_out + b_out              # [B, VOCAB]

Distribution: the recurrence (small, strictly sequential) is replicated on all
8 cores; the output projection and W_out are sharded over the vocab dimension
(4000 columns per core, padded to 4096).

Device program (SPMD, identical on every core except the W_out shard):
  phase 0: F = x@W_lin + b_lin (+b_w variant), S = sigmoid(F), G = x@W_ll +
           b_ll + b_lw for all T*B rows -> DRAM scratch (bf16 matmuls).
  phase A: 80 sequential CARU steps.  h lives in a packed [128, 512] layout
           (partition = 4 hid-chunks x 32 batch, free = 512 hid).  The two
           [32,2048]x[2048,2048] matmuls per step use h^T tiles [128, 32] as
           the PE stationary operand, 4-way column-packed over the PE array
           (tile_position), with bf16 [W_w | W_lw] as the moving operand.
           TensorE transposes rebuild h^T after each update; h^T (bf16) is
           streamed to DRAM for phase B.
  phase B: projection H[2560,2048] @ W_out[2048, 4096-shard].  Stationary =
           h^T chunk tiles [128, 128], moving = resident bf16 W_out, 8 PSUM
           banks accumulating over the 16 K-tiles.
"""

import os
import sys

import numpy as np

VOCAB, EMB, HID, T, B = 32000, 256, 2048, 80, 32
NCORES = 8
VSH = VOCAB // NCORES  # 4000
VPAD = 4096
UNK = 0


def _ensure_concourse():
    try:
        import concourse.bass  # noqa: F401
        return
    except ImportError:
        pass
    for p in ("/opt/trn_rl_repo", "/root/.axon_site/_ro/trn_rl_repo"):
        if os.path.isdir(p) and p not in sys.path:
            sys.path.insert(0, p)
    import concourse.bass  # noqa: F401


def build_decoder(nc, tc, Tn, vpad, has_bias, has_bw):
    """Emit the decoder program into TileContext tc. Returns nothing; tensors
    are declared by name (see in-map construction below)."""
    from contextlib import ExitStack

    import concourse.mybir as mybir

    f32 = mybir.dt.float32
    bf16 = mybir.dt.bfloat16
    AF = mybir.ActivationFunctionType

    rows = Tn * 32
    assert rows % 128 == 0
    n_rt = rows // 128
    KE = EMB // 128  # k-tiles over EMB (2)
    NCH = HID // 512  # 512-chunks over HID (4)
    KT = HID // 128  # k-tiles over HID (16)
    NB = vpad // 512  # psum banks in phase B
    assert NB <= 8
    cs = 16 if Tn % 16 == 0 else Tn  # chunk steps for phase B
    assert Tn % cs == 0 and cs % 4 == 0
    n_ck = Tn // cs
    n_mt = cs // 4  # 128-row tiles per chunk

    xT = nc.dram_tensor("x_T", [EMB, rows], bf16, kind="ExternalInput").ap()
    wlin = nc.dram_tensor("W_lin", [EMB, HID], bf16, kind="ExternalInput").ap()
    wll = nc.dram_tensor("W_ll", [EMB, HID], bf16, kind="ExternalInput").ap()
    hpk = nc.dram_tensor("h_pack", [128, 512], f32, kind="ExternalInput").ap()
    ht0 = nc.dram_tensor("hT0", [128, 512], bf16, kind="ExternalInput").ap()
    wc = nc.dram_tensor("Wc", [HID, 2 * HID], bf16, kind="ExternalInput").ap()
    wout = nc.dram_tensor("Wout", [HID, vpad], bf16, kind="ExternalInput").ap()
    idin = nc.dram_tensor("ident", [128, 128], f32, kind="ExternalInput").ap()
    bia = nc.dram_tensor("biasv", [3, HID], bf16, kind="ExternalInput").ap()
    out = nc.dram_tensor("out", [rows, vpad], f32, kind="ExternalOutput").ap()

    with ExitStack() as ctx:
        constp = ctx.enter_context(tc.tile_pool(name="const", bufs=1))
        dramp = ctx.enter_context(tc.tile_pool(name="drsc", bufs=1, space="DRAM"))

        ident = constp.tile([128, 128], f32)
        nc.sync.dma_start(ident[:], idin)
        zeros_bf = constp.tile([128, 128], bf16)
        nc.vector.memset(zeros_bf[:], 0.0)
        ones_sb = constp.tile([1, 128], bf16)
        bias_sb = constp.tile([3, HID], bf16)
        if has_bias:
            nc.vector.memset(ones_sb[:], 1.0)
            nc.sync.dma_start(bias_sb[:], bia)

        F2d = dramp.tile([rows, HID], f32)
        Sd = dramp.tile([rows, HID], f32)
        Gd = dramp.tile([rows, HID], f32)
        HTd = dramp.tile([Tn, 128, 512], bf16)

        # ---------------- phase 0: F2/S/G precompute ----------------
        with tc.tile_pool(name="p0w", bufs=1) as p0w, \
                tc.tile_pool(name="p0io", bufs=4) as p0io, \
                tc.tile_pool(name="p0ps", bufs=4, space="PSUM") as p0ps:
            xTs = p0w.tile([128, KE * rows], bf16)
            for k in range(KE):
                nc.sync.dma_start(
                    xTs[:, k * rows:(k + 1) * rows], xT[k * 128:(k + 1) * 128, :])
            wls = p0w.tile([128, KE * HID], bf16)
            wlls = p0w.tile([128, KE * HID], bf16)
            for k in range(KE):
                nc.sync.dma_start(
                    wls[:, k * HID:(k + 1) * HID], wlin[k * 128:(k + 1) * 128, :])
                nc.sync.dma_start(
                    wlls[:, k * HID:(k + 1) * HID], wll[k * 128:(k + 1) * 128, :])

            for rt in range(n_rt):
                for ncx in range(NCH):
                    sl = slice(ncx * 512, ncx * 512 + 512)
                    psF = p0ps.tile([128, 512], f32, tag="psF")
                    psG = p0ps.tile([128, 512], f32, tag="psG")
                    for k in range(KE):
                        lh = xTs[:, k * rows + rt * 128:
                                 k * rows + rt * 128 + 128]
                        nc.tensor.matmul(
                            psF[:], lh,
                            wls[:, k * HID + ncx * 512:
                                k * HID + ncx * 512 + 512],
                            start=(k == 0), stop=(k == KE - 1 and not has_bias))
                        nc.tensor.matmul(
                            psG[:], lh,
                            wlls[:, k * HID + ncx * 512:
                                 k * HID + ncx * 512 + 512],
                            start=(k == 0), stop=(k == KE - 1 and not has_bias))
                    if has_bias:
                        nc.tensor.matmul(
                            psF[:], ones_sb[0:1, :],
                            bias_sb[0:1, sl],
                            start=False, stop=True)
                        nc.tensor.matmul(
                            psG[:], ones_sb[0:1, :],
                            bias_sb[2:3, sl],
                            start=False, stop=True)
                    S_t = p0io.tile([128, 512], f32, tag="S")
                    nc.scalar.activation(S_t[:], psF[:], AF.Sigmoid)
                    if has_bw:
                        # F2 = F + b_w (tanh input); must come after S reads psF
                        nc.tensor.matmul(
                            psF[:], ones_sb[0:1, :],
                            bias_sb[1:2, sl],
                            start=False, stop=True, skip_group_check=True)
                    F2_t = p0io.tile([128, 512], f32, tag="F2")
                    nc.vector.tensor_copy(F2_t[:], psF[:])
                    G_t = p0io.tile([128, 512], f32, tag="G")
                    nc.vector.tensor_copy(G_t[:], psG[:])
                    rsl = slice(rt * 128, rt * 128 + 128)
                    nc.sync.dma_start(Sd[rsl, sl], S_t[:])
                    nc.sync.dma_start(F2d[rsl, sl], F2_t[:])
                    nc.sync.dma_start(Gd[rsl, sl], G_t[:])

        # ---------------- phase A: recurrence ----------------
        with tc.tile_pool(name="wcp", bufs=1) as wcp, \
                tc.tile_pool(name="hp", bufs=2) as hp, \
                tc.tile_pool(name="htp", bufs=2) as htp, \
                tc.tile_pool(name="stp", bufs=3) as stp, \
                tc.tile_pool(name="ewp", bufs=2) as ewp, \
                tc.tile_pool(name="apsp", bufs=2, space="PSUM") as apsp, \
                tc.tile_pool(name="trpp", bufs=2, space="PSUM") as trpp:
            wcs = [wcp.tile([128, 8 * 4096], bf16, tag=f"wc{i}", name=f"wc{i}")
                   for i in range(2)]
            for kt in range(KT):
                nc.sync.dma_start(
                    wcs[kt // 8][:, (kt % 8) * 4096:(kt % 8 + 1) * 4096],
                    wc[kt * 128:(kt + 1) * 128, :])

            h_cur = hp.tile([128, 512], f32, tag="h")
            nc.sync.dma_start(h_cur[:], hpk)
            hT_cur = htp.tile([128, 512], bf16, tag="hT")
            nc.sync.dma_start(hT_cur[:], ht0)

            for t in range(Tn):
                F2_t = stp.tile([128, 512], f32, tag="f2")
                S_t = stp.tile([128, 512], f32, tag="s")
                G_t = stp.tile([128, 512], f32, tag="g")
                src = slice(t * 32, (t + 1) * 32)
                nc.sync.dma_start(
                    F2_t[:], F2d[src, :].rearrange("b (c f) -> c b f", c=4))
                nc.sync.dma_start(
                    S_t[:], Sd[src, :].rearrange("b (c f) -> c b f", c=4))
                nc.sync.dma_start(
                    G_t[:], Gd[src, :].rearrange("b (c f) -> c b f", c=4))

                ps0 = apsp.tile([128, 512], f32, tag="ps0")
                ps1 = apsp.tile([128, 512], f32, tag="ps1")
                # Initialize both banks with a full-partition zero matmul so the
                # partition-sliced column-group matmuls can all accumulate
                # (start=False) — safe under both whole-bank and per-partition
                # has_written-clear semantics.
                # Gate bank (ps1) fully first, then candidate bank (ps0):
                # the gate elementwise chain (sigmoid/l/(1-l)h) overlaps the
                # second half of the PE matmul work instead of serializing
                # after all of it.
                nc.tensor.matmul(ps1[:], zeros_bf[:], wcs[0][:, 0:512],
                                 start=True, stop=False)
                for kt in range(KT):
                    off = 128 * (kt % 4) + 32 * (kt // 4)
                    lh = hT_cur[:, off:off + 32]
                    wt = wcs[kt // 8]
                    base = (kt % 8) * 4096
                    for g in range(4):
                        gs = slice(32 * g, 32 * g + 32)
                        nc.tensor.matmul(
                            ps1[gs, :], lh,
                            wt[:, base + 2048 + g * 512:base + 2048 + g * 512 + 512],
                            start=False, stop=False,
                            tile_position=(0, 32 * g))
                # Close the group with a full-partition zero matmul
                # (region-global group bookkeeping; adds 0 to the data).
                nc.tensor.matmul(ps1[:], zeros_bf[:], wcs[0][:, 0:512],
                                 start=False, stop=True)
                nc.tensor.matmul(ps0[:], zeros_bf[:], wcs[0][:, 0:512],
                                 start=True, stop=False)
                for kt in range(KT):
                    off = 128 * (kt % 4) + 32 * (kt // 4)
                    lh = hT_cur[:, off:off + 32]
                    wt = wcs[kt // 8]
                    base = (kt % 8) * 4096
                    for g in range(4):
                        gs = slice(32 * g, 32 * g + 32)
                        nc.tensor.matmul(
                            ps0[gs, :], lh,
                            wt[:, base + g * 512:base + g * 512 + 512],
                            start=False, stop=False,
                            tile_position=(0, 32 * g))
                nc.tensor.matmul(ps0[:], zeros_bf[:], wcs[0][:, 0:512],
                                 start=False, stop=True)

                tmp1 = ewp.tile([128, 512], f32, tag="t1")
                nc.vector.tensor_add(tmp1[:], ps1[:], G_t[:])
                s1 = ewp.tile([128, 512], f32, tag="s1")
                nc.scalar.activation(s1[:], tmp1[:], AF.Sigmoid)
                l_ = ewp.tile([128, 512], f32, tag="l")
                nc.vector.tensor_mul(l_[:], S_t[:], s1[:])
                lh_ = ewp.tile([128, 512], f32, tag="lh")
                nc.vector.tensor_mul(lh_[:], l_[:], h_cur[:])
                q_ = ewp.tile([128, 512], f32, tag="q")
                nc.vector.tensor_sub(q_[:], h_cur[:], lh_[:])
                tmp0 = ewp.tile([128, 512], f32, tag="t0")
                nc.vector.tensor_add(tmp0[:], ps0[:], F2_t[:])
                n_ = ewp.tile([128, 512], f32, tag="n")
                nc.scalar.activation(n_[:], tmp0[:], AF.Tanh)
                p_ = ewp.tile([128, 512], f32, tag="p")
                nc.vector.tensor_mul(p_[:], l_[:], n_[:])
                h_new = hp.tile([128, 512], f32, tag="h")
                nc.vector.tensor_add(h_new[:], q_[:], p_[:])

                ptr = trpp.tile([128, 512], f32, tag="tr")
                for q4 in range(4):
                    nc.tensor.transpose(
                        ptr[:, 128 * q4:128 * q4 + 128],
                        h_new[:, 128 * q4:128 * q4 + 128], ident[:])
                hT_new = htp.tile([128, 512], bf16, tag="hT")
                nc.vector.tensor_copy(hT_new[:], ptr[:])
                nc.sync.dma_start(HTd[t], hT_new[:])

                h_cur, hT_cur = h_new, hT_new

        # ---------------- phase B: output projection ----------------
        with tc.tile_pool(name="wop", bufs=1) as wop, \
                tc.tile_pool(name="htcp", bufs=2) as htcp, \
                tc.tile_pool(name="obp", bufs=12) as obp, \
                tc.tile_pool(name="bpsp", bufs=NB, space="PSUM") as bpsp:
            wos = [wop.tile([128, 8 * vpad], bf16, tag=f"wo{i}", name=f"wo{i}")
                   for i in range(2)]
            for kt in range(KT):
                nc.sync.dma_start(
                    wos[kt // 8][:, (kt % 8) * vpad:(kt % 8 + 1) * vpad],
                    wout[kt * 128:(kt + 1) * 128, :])
            for ck in range(n_ck):
                # Reorder h^T into K-tile-major contiguous layout: column
                # kt*cs*32 + s*32 + b  <-  HTd[ck*cs+s, p, 128*(kt%4)+32*(kt//4)+b]
                htc = htcp.tile([128, KT * cs * 32], bf16, tag="htc")
                for kt in range(KT):
                    off = 128 * (kt % 4) + 32 * (kt // 4)
                    nc.sync.dma_start(
                        htc[:, kt * cs * 32:(kt + 1) * cs * 32],
                        HTd[ck * cs:(ck + 1) * cs, :, off:off + 32]
                        .rearrange("s p b -> p s b"))
                for mt in range(n_mt):
                    pns = [bpsp.tile([128, 512], f32, tag="pn", name=f"pn{n}")
                           for n in range(NB)]
                    for kt in range(KT):
                        lh = htc[:, kt * cs * 32 + mt * 128:
                                 kt * cs * 32 + mt * 128 + 128]
                        wtile = wos[kt // 8]
                        base = (kt % 8) * vpad
                        for n in range(NB):
                            nc.tensor.matmul(
                                pns[n][:], lh,
                                wtile[:, base + n * 512:base + (n + 1) * 512],
                                start=(kt == 0), stop=(kt == KT - 1))
                    row0 = ck * cs * 32 + mt * 128
                    for n in range(NB):
                        ob = obp.tile([128, 512], f32, tag="ob")
                        nc.vector.tensor_copy(ob[:], pns[n][:])
                        nc.sync.dma_start(
                            out[row0:row0 + 128, n * 512:(n + 1) * 512], ob[:])


def host_prepare(hidden, trg, emb, W_lin, b_lin, W_w, b_w, W_lw, b_lw, W_ll,
                 b_ll, W_out, b_out, Tn=T, vpad=VPAD, ncores=NCORES):
    """Build the per-core input maps (numpy only: gather, transpose, pack)."""
    import ml_dtypes

    bf16 = ml_dtypes.bfloat16
    trg = np.asarray(trg)
    words = np.concatenate(
        [np.full((1, trg.shape[1]), UNK, dtype=trg.dtype), trg[:-1]], axis=0)
    x = np.asarray(emb, np.float32)[words]  # [Tn, B, EMB]
    rows = Tn * B
    xT = np.ascontiguousarray(x.reshape(rows, EMB).T)

    h = np.asarray(hidden, np.float32)
    h_pack = np.ascontiguousarray(
        h.reshape(B, 4, 512).transpose(1, 0, 2).reshape(128, 512))
    hT0 = np.ascontiguousarray(
        h.reshape(B, 4, 4, 128).transpose(3, 2, 1, 0).reshape(128, 512)
    ).astype(bf16)

    Wc = np.concatenate(
        [np.asarray(W_w, np.float32), np.asarray(W_lw, np.float32)],
        axis=1).astype(bf16)
    biasv = np.stack([
        np.asarray(b_lin, np.float32),
        np.asarray(b_w, np.float32),
        np.asarray(b_lw, np.float32) + np.asarray(b_ll, np.float32)])
    ident = np.eye(128, dtype=np.float32)

    common = {
        "x_T": xT.astype(bf16),
        "W_lin": np.ascontiguousarray(np.asarray(W_lin, np.float32)).astype(bf16),
        "W_ll": np.ascontiguousarray(np.asarray(W_ll, np.float32)).astype(bf16),
        "h_pack": h_pack,
        "hT0": hT0,
        "Wc": Wc,
        "ident": ident,
        "biasv": biasv.astype(bf16),
    }
    Wo = np.asarray(W_out, np.float32)
    vsh = Wo.shape[1] // ncores
    in_maps = []
    for j in range(ncores):
        wo_j = np.zeros((HID, vpad), dtype=bf16)
        wo_j[:, :vsh] = Wo[:, j * vsh:(j + 1) * vsh].astype(bf16)
        m = dict(common)
        m["Wout"] = wo_j
        in_maps.append(m)
    has_bias = bool(np.any(biasv))
    has_bw = bool(np.any(np.asarray(b_w)))
    return in_maps, has_bias, has_bw, vsh


_CACHE = {}


def compile_decoder(Tn, vpad, has_bias, has_bw, ncores):
    key = (Tn, vpad, has_bias, has_bw, ncores)
    if key in _CACHE:
        return _CACHE[key]
    _ensure_concourse()
    import concourse.bacc as bacc
    import concourse.tile as tile

    nc = bacc.Bacc("TRN2", target_bir_lowering=False, debug=False,
                   num_devices=ncores)
    with tile.TileContext(nc) as tc:
        build_decoder(nc, tc, Tn, vpad, has_bias, has_bw)
    nc.compile()
    _CACHE[key] = nc
    return nc


def kernel(hidden, trg, emb, W_lin, b_lin, W_w, b_w, W_lw, b_lw, W_ll, b_ll,
           W_out, b_out, _trace=False):
    _ensure_concourse()
    from concourse.bass_utils import run_bass_kernel_spmd

    in_maps, has_bias, has_bw, vsh = host_prepare(
        hidden, trg, emb, W_lin, b_lin, W_w, b_w, W_lw, b_lw, W_ll, b_ll,
        W_out, b_out)
    nc = compile_decoder(T, VPAD, has_bias, has_bw, NCORES)
    res = run_bass_kernel_spmd(
        nc, in_maps, core_ids=list(range(NCORES)), trace=_trace)
    parts = [res.results[j]["out"][:, :vsh] for j in range(NCORES)]
    full = np.concatenate(parts, axis=1).reshape(T, B, VOCAB)
    b_out = np.asarray(b_out, np.float32)
    if np.any(b_out):
        full = full + b_out
    if _trace:
        kernel._last_results = res
    return np.ascontiguousarray(full.astype(np.float32))
